# revision 35
# baseline (speedup 1.0000x reference)
"""HQQ-compatible 4-bit quantized linear layer on 8 Trainium2 NeuronCores.

Problem: y = x @ W.T + bias where W = ((unpack4(W_q) - zero) * scale).reshape(8192, 8192)
  x: (64, 8192) f32; W_q: (32, 1048576) int32 (bytes, two nibbles packed);
  scale/zero: (1, 1048576) f32; bias: (8192,) f32.

Math per output element (OUT=IN=8192, GS=64, NG=2**20):
  W[o, i] = (Wu[gs, ng] - zero[ng]) * scale[ng],  gs = o // 128, ng = (o % 128)*8192 + i
  Wu[r, ng] = W_q[r, ng] >> 4 (r < 32) | W_q[r-32, ng] & 0xF (r >= 32).

Sharding (tensor-parallel over output features, by ng blocks):
  core m owns ng in [m*131072, (m+1)*131072)  <=>  (o % 128) in [m*16, m*16+16).
  core m computes the 1024 outputs o = gs*128 + m*16 + b (gs in [0,64), b in [0,16)).

Per-core device pipeline (linearity: y = sum x*sc*Wu - sum x*(sc*zero) + bias):
  - host splits W_q bytes into hi/lo nibble u8 arrays (bit repacking only),
    laid out as contiguous per-pair-group DRAM blocks [(pg p), cols] so each
    chunk DMA reads DRAM sequentially
  - hi: HWDGE on the sync ring (dedicated to the 4.2MB nibble stream),
    ScalarE activation-copy casts u8 -> bf16 at half-group grain
  - lo: SWDGE (gpsimd ring) casts u8 -> bf16 in-flight, one 1MB-write
    dispatch per pair group; bias rides this ring at the tail (only needed
    at the epilogue)
  - consts (scale, scale*zero, x) ride the scalar-engine HWDGE ring; their
    dispatches overlap the wait for the first hi chunk
  - VectorE: one tensor_tensor mult per (nibble, 8-k pair group): bf16
    nibbles times scale broadcast over r (2x DVE mode; b-minor unit stride);
    first and last pair groups run at 4-k grain for faster pipeline fill and
    a shorter tail quantum
  - TensorE: per k two N=512 matmuls (hi|lo) + one N=16 matmul (sc*zero term),
    all accumulating over the 64 k-tiles in PSUM
  - epilogue: tmp = psC_bc - bias (one TT), y = psW - tmp (one TT), DMA out
    on the sync ring

Measured engine loads per core (NTFF): DVE ~39.5us (the 8.4M-element scale
multiply is its 34.2us floor at TT-bf16 2x mode), ScalarE ~33.5us, TensorE
~33us, DMA ~25MB combined at an effective 340-520 GB/s under 3-queue
contention, plus ~14us fixed framework pre/postamble inside the measured
window.  These are mutually balanced; the kernel sits at the practical
plateau of this decomposition (~70-72us).
"""

import ml_dtypes
import numpy as np

OUT = 8192
IN = 8192
GS = 64
NG = OUT * IN // GS  # 1048576
B = 64
NCORES = 8
NGC = NG // NCORES   # 131072 groups per core
BB = 16              # width of the (o % 128) block per core
KT = IN // 128       # 64 in-tiles of 128
CK = 4               # k-tiles per chunk
NCH = KT // CK       # 16 chunks
PK0 = 2 * CK         # k-tiles per pair-group (DMA/TT grain)

_CACHE = {}


def _build_nc():
    import concourse.bacc as bacc
    import concourse.mybir as mybir
    import concourse.tile as tile
    from concourse.alu_op_type import AluOpType

    f16 = mybir.dt.bfloat16
    f32 = mybir.dt.float32
    u8 = mybir.dt.uint8

    nc = bacc.Bacc(None, target_bir_lowering=False, debug=False)

    NPG = NCH // 2  # pair-groups
    xt_d = nc.dram_tensor("xt", [128, KT * B], f16, kind="ExternalInput")
    # nibble streams laid out as contiguous per-pair-group blocks so every
    # chunk DMA reads DRAM fully sequentially (strided 4KB segments measurably
    # throttle HBM)
    hi_d = nc.dram_tensor("hi", [NPG * 128, PK0 * 512], u8, kind="ExternalInput")
    lo_d = nc.dram_tensor("lo", [NPG * 128, PK0 * 512], u8, kind="ExternalInput")
    sc_d = nc.dram_tensor("sc", [128, KT * BB], f16, kind="ExternalInput")
    sz_d = nc.dram_tensor("sz", [128, KT * BB], f16, kind="ExternalInput")
    bs_d = nc.dram_tensor("bs", [2, 512], f32, kind="ExternalInput")
    # output as [ (h t), 512 ]: rows 0:64 hi-half tokens, rows 64:128 lo-half
    y_d = nc.dram_tensor("y", [2 * B, 512], f32, kind="ExternalOutput")

    with tile.TileContext(nc) as tc:
        with (
            tc.tile_pool(name="const", bufs=1) as cpool,
            tc.tile_pool(name="wq", bufs=6) as wqpool,
            tc.tile_pool(name="nibhi", bufs=3) as hipool,
            tc.tile_pool(name="niblo", bufs=3) as lopool,
            tc.tile_pool(name="ws", bufs=3) as wspool,
            tc.tile_pool(name="psum", bufs=1, space="PSUM") as pspool,
            tc.tile_pool(name="outp", bufs=1) as opool,
        ):
            # consts on the scalar-engine HWDGE ring (parallel to sync ring);
            # small ones first so they clear the ring before the 1MB xt
            sc_sb = cpool.tile([128, KT * BB], f16)
            nc.scalar.dma_start(out=sc_sb[:], in_=sc_d[:])
            sz_sb = cpool.tile([128, KT * BB], f16)
            nc.scalar.dma_start(out=sz_sb[:], in_=sz_d[:])
            xt_sb = cpool.tile([128, KT * B], f16)
            nc.scalar.dma_start(out=xt_sb[:], in_=xt_d[:])
            bias_sb = cpool.tile([2 * B, 512], f32)

            # PE column tiling: the hi stream computes on array columns 0:63
            # (PSUM partitions 0:63), the lo stream on columns 64:127 — the
            # two N=512 matmuls per k-tile run CONCURRENTLY (tile_position is
            # auto-derived from the PSUM slice's base partition), halving the
            # tensor-engine streaming time.
            psW = pspool.tile([2 * B, 512], f32)  # rows 0:64 hi, 64:128 lo
            psC = pspool.tile([B, BB], f32)       # zero-term

            PK = PK0             # k-tiles per TT/matmul pair-group
            cw = CK * 512
            tiles = {}

            def sc_view(ka, kb):
                return (
                    sc_sb[:, ka * BB : kb * BB]
                    .rearrange("p (k b) -> p k b", b=BB)
                    .unsqueeze(2)
                    .broadcast_to((128, kb - ka, 32, BB))
                )

            def emit_tt(p, stream, spans):
                hi_f, lo_t, ws = tiles[p]
                src = hi_f if stream == "hi" else lo_t
                col0 = 0 if stream == "hi" else 512
                ws4 = ws[:].rearrange("p (k n) -> p k n", n=1024)
                for (ka, kb) in spans:
                    nc.vector.tensor_tensor(
                        out=ws4[:, ka:kb, col0 : col0 + 512].rearrange(
                            "p k (r b) -> p k r b", b=BB
                        ),
                        in0=src[:, ka * 512 : kb * 512].rearrange(
                            "p (k r b) -> p k r b", k=kb - ka, b=BB
                        ),
                        in1=sc_view(p * PK + ka, p * PK + kb),
                        op=AluOpType.mult,
                    )

            def emit_mms(p):
                ws4 = tiles[p][2][:].rearrange("p (k n) -> p k n", n=1024)
                for kl in range(PK):
                    k = p * PK + kl
                    lhsT = xt_sb[:, k * B : (k + 1) * B]
                    first = k == 0
                    last_k = k == KT - 1
                    nc.tensor.matmul(
                        psW[0:B, :], lhsT, ws4[:, kl, 0:512],
                        start=first, stop=last_k,
                    )
                    nc.tensor.matmul(
                        psW[B : 2 * B, :], lhsT, ws4[:, kl, 512:1024],
                        start=first, stop=last_k, tile_position=(0, 64),
                    )
                    nc.tensor.matmul(
                        psC[:], lhsT, sz_sb[:, k * BB : (k + 1) * BB],
                        start=first, stop=last_k,
                    )

            for pg in range(NCH // 2):
                k0 = pg * PK
                hi_f = hipool.tile([128, PK * 512], f16, tag="hi_f")
                # hi: one contiguous-block DMA per pair group on the sync
                # HWDGE ring, ScalarE casts at CK grain
                hi_u8 = wqpool.tile([128, PK * 512], u8, tag="hi_u8")
                nc.sync.dma_start(
                    out=hi_u8[:], in_=hi_d[pg * 128 : (pg + 1) * 128, :]
                )
                # first/last pair group cast at CK grain (pipeline fill /
                # short tail quantum); the steady-state middle uses one big
                # ACTIVATE per pair group — the 224-cycle per-instruction
                # overhead is what paces the hi-side mid-run
                if pg == 0 or pg == NCH // 2 - 1:
                    for half in range(2):
                        nc.scalar.activation(
                            out=hi_f[:, half * cw : (half + 1) * cw],
                            in_=hi_u8[:, half * cw : (half + 1) * cw],
                            func=mybir.ActivationFunctionType.Copy, scale=1.0,
                        )
                else:
                    nc.scalar.activation(
                        out=hi_f[:], in_=hi_u8[:],
                        func=mybir.ActivationFunctionType.Copy, scale=1.0,
                    )
                # lo: one contiguous-block SWDGE cast-DMA per pair group
                lo_t = lopool.tile([128, PK * 512], f16, tag="lo_f")
                lo_off = 0
                nc.gpsimd.dma_start(
                    out=lo_t[:], in_=lo_d[pg * 128 : (pg + 1) * 128, :]
                )

                ws = wspool.tile([128, PK * 1024], f16, tag="ws")
                tiles[pg] = (hi_f, lo_t, ws)

                # hi-TT for this pair group runs now; the lo-TT and matmuls
                # for the PREVIOUS pair group are emitted behind it.  The
                # hi side (u8 stream + ScalarE cast) is always ready one pg
                # ahead of the heavier lo cast-DMA, so this keeps the DVE off
                # the lo arrival latency — and after the LAST lo chunk lands,
                # only its own lo-TT remains instead of a full hi+lo pair.
                emit_tt(pg, "hi", [(0, CK), (CK, PK)] if pg == 0 else [(0, PK)])
                if pg > 0:
                    emit_tt(pg - 1, "lo", [(0, PK)])
                    emit_mms(pg - 1)
            last = NCH // 2 - 1
            emit_tt(last, "lo", [(0, CK), (CK, PK)])
            emit_mms(last)

            # bias arrives on the gpsimd ring after the lo stream (it is only
            # needed here, ~35us in): hi-half rows 0:64, lo-half rows 64:128
            nc.gpsimd.dma_start(
                out=bias_sb[0:B, :], in_=bs_d[0:1, :].broadcast_to((B, 512))
            )
            nc.gpsimd.dma_start(
                out=bias_sb[B : 2 * B, :],
                in_=bs_d[1:2, :].broadcast_to((B, 512)),
            )

            out_sb = opool.tile([2 * B, 512], f32)
            tmp_sb = opool.tile([2 * B, 512], f32)
            psC_sb = opool.tile([2 * B, BB], f32)
            nc.scalar.copy(out=psC_sb[0:B, :], in_=psC[:])
            # the zero-term is shared by both halves (it does not depend on
            # gs); engines are partition-lockstep, so duplicate it to the
            # lo-half partitions with a tiny SBUF->SBUF DMA
            nc.sync.dma_start(out=psC_sb[B : 2 * B, :], in_=psC_sb[0:B, :])
            # tmp = psC (broadcast over g) - bias;  y = psW - tmp
            # (both on DVE: GpSimd compute steals the shared SBUF port and
            # halves the throughput of concurrent DVE tensor_tensor ops)
            nc.vector.tensor_tensor(
                out=tmp_sb[:].rearrange("p (g b) -> p g b", b=BB),
                in0=psC_sb[:].unsqueeze(1).broadcast_to((2 * B, GS // 2, BB)),
                in1=bias_sb[:].rearrange("p (g b) -> p g b", b=BB),
                op=AluOpType.subtract,
            )
            nc.vector.tensor_tensor(
                out=out_sb[:], in0=psW[:], in1=tmp_sb[:], op=AluOpType.subtract
            )
            nc.sync.dma_start(out=y_d[:], in_=out_sb[:])

    nc.compile()
    return nc


def _get_nc():
    if "nc" not in _CACHE:
        _CACHE["nc"] = _build_nc()
    return _CACHE["nc"]


def _prep_inputs(x, W_q, scale, zero, bias):
    """Host-side shard + layout prep (dtype narrowing / bit repack / transposes)."""
    xt = (
        x.T.reshape(KT, 128, B).transpose(1, 0, 2).reshape(128, KT * B)
    ).astype(ml_dtypes.bfloat16)  # (p, (k t))
    wq_u8 = W_q.astype(np.uint8)
    hi_u8 = (wq_u8 >> 4).astype(np.uint8)
    lo_u8 = (wq_u8 & 0xF).astype(np.uint8)
    sz_full = (scale.astype(np.float64) * zero.astype(np.float64)).astype(np.float32)

    def wlayout(arr_m):
        # arr_m: (32, NGC) one core's nibble slice -> contiguous per-pair-
        # group blocks [(pg p), (kl, r, b)] so each chunk DMA reads DRAM
        # sequentially
        a = arr_m.reshape(32, BB, IN)          # (r, b, in)
        a = a.transpose(2, 0, 1)               # (in, r, b): col = r*16+b
        a = a.reshape(KT, 128, 512)            # (k, p, rb)
        a = a.transpose(1, 0, 2)               # (p, k, rb)
        a = a.reshape(128, KT // PK0, PK0 * 512)  # (p, pg, cols)
        a = a.transpose(1, 0, 2)               # (pg, p, cols)
        return np.ascontiguousarray(a.reshape((KT // PK0) * 128, PK0 * 512))

    in_maps = []
    for m in range(NCORES):
        sl = slice(m * NGC, (m + 1) * NGC)
        sc_m = (
            scale[0, sl]
            .reshape(BB, IN)
            .T.reshape(KT, 128, BB)
            .transpose(1, 0, 2)
            .reshape(128, KT * BB)
        ).astype(ml_dtypes.bfloat16)
        sz_m = (
            sz_full[0, sl]
            .reshape(BB, IN)
            .T.reshape(KT, 128, BB)
            .transpose(1, 0, 2)
            .reshape(128, KT * BB)
        ).astype(ml_dtypes.bfloat16)
        # out (row h*64+t, col r*16+b)  <->  global out o = (h*32+r)*128 + m*16 + b
        bs_m = (
            bias.reshape(GS, 128)[:, m * BB : (m + 1) * BB]  # (gs, b)
            .reshape(2, 512)
            .astype(np.float32)
        )
        in_maps.append(
            {
                "xt": xt,
                "hi": wlayout(hi_u8[:, sl]),
                "lo": wlayout(lo_u8[:, sl]),
                "sc": np.ascontiguousarray(sc_m),
                "sz": np.ascontiguousarray(sz_m),
                "bs": bs_m,
            }
        )
    return in_maps


def _gather(results):
    ybig = np.stack([results[m]["y"] for m in range(NCORES)], axis=0)  # (m, 2B, 512)
    ybig = ybig.reshape(NCORES, 2, B, 32, BB)  # (m, h, t, r, b)
    return np.ascontiguousarray(
        ybig.transpose(2, 1, 3, 0, 4).reshape(B, OUT)
    )  # o = (h*32+r)*128 + m*16 + b


def run_on_hw(x, W_q, scale, zero, bias, trace=False, **trace_kw):
    """Returns (y_full, BassKernelResults)."""
    from concourse.bass_utils import run_bass_kernel_spmd

    nc = _get_nc()
    in_maps = _prep_inputs(x, W_q, scale, zero, bias)
    res = run_bass_kernel_spmd(
        nc, in_maps, list(range(NCORES)), trace=trace, **trace_kw
    )
    return _gather(res.results), res


def kernel(x, W_q, scale, zero, bias):
    y, _ = run_on_hw(x, W_q, scale, zero, bias, trace=False)
    return y


# revision 36
# speedup vs baseline: 1.0410x; 1.0410x over previous
"""HQQ-compatible 4-bit quantized linear layer on 8 Trainium2 NeuronCores.

Problem: y = x @ W.T + bias where W = ((unpack4(W_q) - zero) * scale).reshape(8192, 8192)
  x: (64, 8192) f32; W_q: (32, 1048576) int32 (bytes, two nibbles packed);
  scale/zero: (1, 1048576) f32; bias: (8192,) f32.

Math per output element (OUT=IN=8192, GS=64, NG=2**20):
  W[o, i] = (Wu[gs, ng] - zero[ng]) * scale[ng],  gs = o // 128, ng = (o % 128)*8192 + i
  Wu[r, ng] = W_q[r, ng] >> 4 (r < 32) | W_q[r-32, ng] & 0xF (r >= 32).

Sharding (tensor-parallel over output features, by ng blocks):
  core m owns ng in [m*131072, (m+1)*131072)  <=>  (o % 128) in [m*16, m*16+16).
  core m computes the 1024 outputs o = gs*128 + m*16 + b (gs in [0,64), b in [0,16)).

Per-core device pipeline (linearity: y = sum x*sc*Wu - sum x*(sc*zero) + bias):
  - host splits W_q bytes into hi/lo nibble u8 arrays (bit repacking only),
    laid out as contiguous per-pair-group DRAM blocks [(pg p), cols] so each
    chunk DMA reads DRAM sequentially
  - hi: HWDGE on the sync ring (dedicated to the 4.2MB nibble stream),
    ScalarE activation-copy casts u8 -> bf16 at half-group grain
  - lo: SWDGE (gpsimd ring) casts u8 -> bf16 in-flight, one 1MB-write
    dispatch per pair group; bias rides this ring at the tail (only needed
    at the epilogue)
  - consts (scale, scale*zero, x) ride the scalar-engine HWDGE ring; their
    dispatches overlap the wait for the first hi chunk
  - VectorE: one tensor_tensor mult per (nibble, 8-k pair group): bf16
    nibbles times scale broadcast over r (2x DVE mode; b-minor unit stride);
    first and last pair groups run at 4-k grain for faster pipeline fill and
    a shorter tail quantum
  - TensorE: per k two N=512 matmuls (hi|lo) + one N=16 matmul (sc*zero term),
    all accumulating over the 64 k-tiles in PSUM
  - epilogue: tmp = psC_bc - bias (one TT), y = psW - tmp (one TT), DMA out
    on the sync ring

Measured engine loads per core (NTFF): DVE ~39.5us (the 8.4M-element scale
multiply is its 34.2us floor at TT-bf16 2x mode), ScalarE ~33.5us, TensorE
~33us, DMA ~25MB combined at an effective 340-520 GB/s under 3-queue
contention, plus ~14us fixed framework pre/postamble inside the measured
window.  These are mutually balanced; the kernel sits at the practical
plateau of this decomposition (~70-72us).
"""

import ml_dtypes
import numpy as np

OUT = 8192
IN = 8192
GS = 64
NG = OUT * IN // GS  # 1048576
B = 64
NCORES = 8
NGC = NG // NCORES   # 131072 groups per core
BB = 16              # width of the (o % 128) block per core
KT = IN // 128       # 64 in-tiles of 128
CK = 4               # k-tiles per chunk
NCH = KT // CK       # 16 chunks
PK0 = 2 * CK         # k-tiles per pair-group (DMA/TT grain)

_CACHE = {}


def _build_nc():
    import concourse.bacc as bacc
    import concourse.mybir as mybir
    import concourse.tile as tile
    from concourse.alu_op_type import AluOpType

    f16 = mybir.dt.bfloat16
    f32 = mybir.dt.float32
    u8 = mybir.dt.uint8

    nc = bacc.Bacc(None, target_bir_lowering=False, debug=False)

    NPG = NCH // 2  # pair-groups
    xt_d = nc.dram_tensor("xt", [128, KT * B], f16, kind="ExternalInput")
    # nibble streams laid out as contiguous per-pair-group blocks so every
    # chunk DMA reads DRAM fully sequentially (strided 4KB segments measurably
    # throttle HBM)
    hi_d = nc.dram_tensor("hi", [NPG * 128, PK0 * 512], u8, kind="ExternalInput")
    lo_d = nc.dram_tensor("lo", [NPG * 128, PK0 * 512], u8, kind="ExternalInput")
    sc_d = nc.dram_tensor("sc", [128, KT * BB], f16, kind="ExternalInput")
    sz_d = nc.dram_tensor("sz", [128, KT * BB], f16, kind="ExternalInput")
    bs_d = nc.dram_tensor("bs", [2, 512], f32, kind="ExternalInput")
    # output as [ (h t), 512 ]: rows 0:64 hi-half tokens, rows 64:128 lo-half
    y_d = nc.dram_tensor("y", [2 * B, 512], f32, kind="ExternalOutput")

    with tile.TileContext(nc) as tc:
        with (
            tc.tile_pool(name="const", bufs=1) as cpool,
            tc.tile_pool(name="wq", bufs=6) as wqpool,
            tc.tile_pool(name="nibhi", bufs=3) as hipool,
            tc.tile_pool(name="niblo", bufs=3) as lopool,
            tc.tile_pool(name="ws", bufs=3) as wspool,
            tc.tile_pool(name="psum", bufs=1, space="PSUM") as pspool,
            tc.tile_pool(name="outp", bufs=1) as opool,
        ):
            # consts on the scalar-engine HWDGE ring (parallel to sync ring);
            # small ones first so they clear the ring before the 1MB xt
            sc_sb = cpool.tile([128, KT * BB], f16)
            nc.scalar.dma_start(out=sc_sb[:], in_=sc_d[:])
            sz_sb = cpool.tile([128, KT * BB], f16)
            nc.scalar.dma_start(out=sz_sb[:], in_=sz_d[:])
            xt_sb = cpool.tile([128, KT * B], f16)
            nc.scalar.dma_start(out=xt_sb[:], in_=xt_d[:])
            bias_sb = cpool.tile([2 * B, 512], f32)

            # PE column tiling: the hi stream computes on array columns 0:63
            # (PSUM partitions 0:63), the lo stream on columns 64:127 — the
            # two N=512 matmuls per k-tile run CONCURRENTLY (tile_position is
            # auto-derived from the PSUM slice's base partition), halving the
            # tensor-engine streaming time.
            psW = pspool.tile([2 * B, 512], f32)  # rows 0:64 hi, 64:128 lo
            psC = pspool.tile([B, BB], f32)       # zero-term

            PK = PK0             # k-tiles per TT/matmul pair-group
            cw = CK * 512
            tiles = {}

            def sc_view(ka, kb):
                return (
                    sc_sb[:, ka * BB : kb * BB]
                    .rearrange("p (k b) -> p k b", b=BB)
                    .unsqueeze(2)
                    .broadcast_to((128, kb - ka, 32, BB))
                )

            def emit_tt(p, stream, spans):
                hi_f, lo_t, ws = tiles[p]
                src = hi_f if stream == "hi" else lo_t
                col0 = 0 if stream == "hi" else 512
                ws4 = ws[:].rearrange("p (k n) -> p k n", n=1024)
                for (ka, kb) in spans:
                    nc.vector.tensor_tensor(
                        out=ws4[:, ka:kb, col0 : col0 + 512].rearrange(
                            "p k (r b) -> p k r b", b=BB
                        ),
                        in0=src[:, ka * 512 : kb * 512].rearrange(
                            "p (k r b) -> p k r b", k=kb - ka, b=BB
                        ),
                        in1=sc_view(p * PK + ka, p * PK + kb),
                        op=AluOpType.mult,
                    )

            def emit_mms(p):
                ws4 = tiles[p][2][:].rearrange("p (k n) -> p k n", n=1024)
                for kl in range(PK):
                    k = p * PK + kl
                    lhsT = xt_sb[:, k * B : (k + 1) * B]
                    first = k == 0
                    last_k = k == KT - 1
                    nc.tensor.matmul(
                        psW[0:B, :], lhsT, ws4[:, kl, 0:512],
                        start=first, stop=last_k,
                    )
                    nc.tensor.matmul(
                        psW[B : 2 * B, :], lhsT, ws4[:, kl, 512:1024],
                        start=first, stop=last_k, tile_position=(0, 64),
                    )
                    nc.tensor.matmul(
                        psC[:], lhsT, sz_sb[:, k * BB : (k + 1) * BB],
                        start=first, stop=last_k,
                    )

            for pg in range(NCH // 2):
                k0 = pg * PK
                hi_f = hipool.tile([128, PK * 512], f16, tag="hi_f")
                # hi: one contiguous-block DMA per pair group on the sync
                # HWDGE ring, ScalarE casts at CK grain
                hi_u8 = wqpool.tile([128, PK * 512], u8, tag="hi_u8")
                nc.sync.dma_start(
                    out=hi_u8[:], in_=hi_d[pg * 128 : (pg + 1) * 128, :]
                )
                # first/last pair group cast at CK grain (pipeline fill /
                # short tail quantum); the steady-state middle uses one big
                # ACTIVATE per pair group — the 224-cycle per-instruction
                # overhead is what paces the hi-side mid-run
                if pg == 0 or pg == NCH // 2 - 1:
                    for half in range(2):
                        nc.scalar.activation(
                            out=hi_f[:, half * cw : (half + 1) * cw],
                            in_=hi_u8[:, half * cw : (half + 1) * cw],
                            func=mybir.ActivationFunctionType.Copy, scale=1.0,
                        )
                else:
                    nc.scalar.activation(
                        out=hi_f[:], in_=hi_u8[:],
                        func=mybir.ActivationFunctionType.Copy, scale=1.0,
                    )
                # lo: one contiguous-block SWDGE cast-DMA per pair group
                lo_t = lopool.tile([128, PK * 512], f16, tag="lo_f")
                lo_off = 0
                nc.gpsimd.dma_start(
                    out=lo_t[:], in_=lo_d[pg * 128 : (pg + 1) * 128, :]
                )

                ws = wspool.tile([128, PK * 1024], f16, tag="ws")
                tiles[pg] = (hi_f, lo_t, ws)

                # pg0: TTs at CK grain so the first matmuls start sooner;
                # last pg too, so the tail quantum after the final lo chunk
                # is half as long.  (Do NOT reorder lo-TTs behind the next
                # pg's hi-TT: the DVE queue is strict FIFO, and a hi-TT
                # waiting on its ScalarE cast head-of-line-blocks the ready
                # lo-TT — measured +10us.)
                fine = pg == 0 or pg == NCH // 2 - 1
                tt_spans = [(0, CK), (CK, PK)] if fine else [(0, PK)]
                for sp in tt_spans:
                    emit_tt(pg, "hi", [sp])
                    emit_tt(pg, "lo", [sp])
                emit_mms(pg)

            # bias arrives on the gpsimd ring after the lo stream (it is only
            # needed here, ~35us in): hi-half rows 0:64, lo-half rows 64:128
            nc.gpsimd.dma_start(
                out=bias_sb[0:B, :], in_=bs_d[0:1, :].broadcast_to((B, 512))
            )
            nc.gpsimd.dma_start(
                out=bias_sb[B : 2 * B, :],
                in_=bs_d[1:2, :].broadcast_to((B, 512)),
            )

            out_sb = opool.tile([2 * B, 512], f32)
            tmp_sb = opool.tile([2 * B, 512], f32)
            psC_sb = opool.tile([2 * B, BB], f32)
            nc.scalar.copy(out=psC_sb[0:B, :], in_=psC[:])
            # the zero-term is shared by both halves (it does not depend on
            # gs); engines are partition-lockstep, so duplicate it to the
            # lo-half partitions with a tiny SBUF->SBUF DMA
            nc.sync.dma_start(out=psC_sb[B : 2 * B, :], in_=psC_sb[0:B, :])
            # tmp = psC (broadcast over g) - bias;  y = psW - tmp
            # (both on DVE: GpSimd compute steals the shared SBUF port and
            # halves the throughput of concurrent DVE tensor_tensor ops)
            nc.vector.tensor_tensor(
                out=tmp_sb[:].rearrange("p (g b) -> p g b", b=BB),
                in0=psC_sb[:].unsqueeze(1).broadcast_to((2 * B, GS // 2, BB)),
                in1=bias_sb[:].rearrange("p (g b) -> p g b", b=BB),
                op=AluOpType.subtract,
            )
            nc.vector.tensor_tensor(
                out=out_sb[:], in0=psW[:], in1=tmp_sb[:], op=AluOpType.subtract
            )
            nc.sync.dma_start(out=y_d[:], in_=out_sb[:])

    nc.compile()
    return nc


def _get_nc():
    if "nc" not in _CACHE:
        _CACHE["nc"] = _build_nc()
    return _CACHE["nc"]


def _prep_inputs(x, W_q, scale, zero, bias):
    """Host-side shard + layout prep (dtype narrowing / bit repack / transposes)."""
    xt = (
        x.T.reshape(KT, 128, B).transpose(1, 0, 2).reshape(128, KT * B)
    ).astype(ml_dtypes.bfloat16)  # (p, (k t))
    wq_u8 = W_q.astype(np.uint8)
    hi_u8 = (wq_u8 >> 4).astype(np.uint8)
    lo_u8 = (wq_u8 & 0xF).astype(np.uint8)
    sz_full = (scale.astype(np.float64) * zero.astype(np.float64)).astype(np.float32)

    def wlayout(arr_m):
        # arr_m: (32, NGC) one core's nibble slice -> contiguous per-pair-
        # group blocks [(pg p), (kl, r, b)] so each chunk DMA reads DRAM
        # sequentially
        a = arr_m.reshape(32, BB, IN)          # (r, b, in)
        a = a.transpose(2, 0, 1)               # (in, r, b): col = r*16+b
        a = a.reshape(KT, 128, 512)            # (k, p, rb)
        a = a.transpose(1, 0, 2)               # (p, k, rb)
        a = a.reshape(128, KT // PK0, PK0 * 512)  # (p, pg, cols)
        a = a.transpose(1, 0, 2)               # (pg, p, cols)
        return np.ascontiguousarray(a.reshape((KT // PK0) * 128, PK0 * 512))

    in_maps = []
    for m in range(NCORES):
        sl = slice(m * NGC, (m + 1) * NGC)
        sc_m = (
            scale[0, sl]
            .reshape(BB, IN)
            .T.reshape(KT, 128, BB)
            .transpose(1, 0, 2)
            .reshape(128, KT * BB)
        ).astype(ml_dtypes.bfloat16)
        sz_m = (
            sz_full[0, sl]
            .reshape(BB, IN)
            .T.reshape(KT, 128, BB)
            .transpose(1, 0, 2)
            .reshape(128, KT * BB)
        ).astype(ml_dtypes.bfloat16)
        # out (row h*64+t, col r*16+b)  <->  global out o = (h*32+r)*128 + m*16 + b
        bs_m = (
            bias.reshape(GS, 128)[:, m * BB : (m + 1) * BB]  # (gs, b)
            .reshape(2, 512)
            .astype(np.float32)
        )
        in_maps.append(
            {
                "xt": xt,
                "hi": wlayout(hi_u8[:, sl]),
                "lo": wlayout(lo_u8[:, sl]),
                "sc": np.ascontiguousarray(sc_m),
                "sz": np.ascontiguousarray(sz_m),
                "bs": bs_m,
            }
        )
    return in_maps


def _gather(results):
    ybig = np.stack([results[m]["y"] for m in range(NCORES)], axis=0)  # (m, 2B, 512)
    ybig = ybig.reshape(NCORES, 2, B, 32, BB)  # (m, h, t, r, b)
    return np.ascontiguousarray(
        ybig.transpose(2, 1, 3, 0, 4).reshape(B, OUT)
    )  # o = (h*32+r)*128 + m*16 + b


def run_on_hw(x, W_q, scale, zero, bias, trace=False, **trace_kw):
    """Returns (y_full, BassKernelResults)."""
    from concourse.bass_utils import run_bass_kernel_spmd

    nc = _get_nc()
    in_maps = _prep_inputs(x, W_q, scale, zero, bias)
    res = run_bass_kernel_spmd(
        nc, in_maps, list(range(NCORES)), trace=trace, **trace_kw
    )
    return _gather(res.results), res


def kernel(x, W_q, scale, zero, bias):
    y, _ = run_on_hw(x, W_q, scale, zero, bias, trace=False)
    return y


# revision 38
# speedup vs baseline: 1.1530x; 1.1076x over previous
"""HQQ-compatible 4-bit quantized linear layer on 8 Trainium2 NeuronCores.

Problem: y = x @ W.T + bias where W = ((unpack4(W_q) - zero) * scale).reshape(8192, 8192)
  x: (64, 8192) f32; W_q: (32, 1048576) int32 (bytes, two nibbles packed);
  scale/zero: (1, 1048576) f32; bias: (8192,) f32.

Math per output element (OUT=IN=8192, GS=64, NG=2**20):
  W[o, i] = (Wu[gs, ng] - zero[ng]) * scale[ng],  gs = o // 128, ng = (o % 128)*8192 + i
  Wu[r, ng] = W_q[r, ng] >> 4 (r < 32) | W_q[r-32, ng] & 0xF (r >= 32).

Sharding (tensor-parallel over output features, by ng blocks):
  core m owns ng in [m*131072, (m+1)*131072)  <=>  (o % 128) in [m*16, m*16+16).
  core m computes the 1024 outputs o = gs*128 + m*16 + b (gs in [0,64), b in [0,16)).

Per-core device pipeline (linearity: y = sum x*sc*Wu - sum x*(sc*zero) + bias):
  - host splits W_q bytes into hi/lo nibble u8 arrays (bit repacking only),
    laid out as contiguous per-pair-group DRAM blocks [(pg p), cols] so each
    chunk DMA reads DRAM sequentially
  - hi: HWDGE on the sync ring (dedicated to the 4.2MB nibble stream),
    ScalarE activation-copy casts u8 -> bf16 at half-group grain
  - lo: SWDGE (gpsimd ring) casts u8 -> bf16 in-flight, one 1MB-write
    dispatch per pair group; bias rides this ring at the tail (only needed
    at the epilogue)
  - consts (scale, scale*zero, x) ride the scalar-engine HWDGE ring; their
    dispatches overlap the wait for the first hi chunk
  - VectorE: one tensor_tensor mult per (nibble, 8-k pair group): bf16
    nibbles times scale broadcast over r (2x DVE mode; b-minor unit stride);
    first and last pair groups run at 4-k grain for faster pipeline fill and
    a shorter tail quantum
  - TensorE: per k two N=512 matmuls (hi|lo) + one N=16 matmul (sc*zero term),
    all accumulating over the 64 k-tiles in PSUM
  - epilogue: tmp = psC_bc - bias (one TT), y = psW - tmp (one TT), DMA out
    on the sync ring

Measured engine loads per core (NTFF): DVE ~39.5us (the 8.4M-element scale
multiply is its 34.2us floor at TT-bf16 2x mode), ScalarE ~33.5us, TensorE
~33us, DMA ~25MB combined at an effective 340-520 GB/s under 3-queue
contention, plus ~14us fixed framework pre/postamble inside the measured
window.  These are mutually balanced; the kernel sits at the practical
plateau of this decomposition (~70-72us).
"""

import ml_dtypes
import numpy as np

OUT = 8192
IN = 8192
GS = 64
NG = OUT * IN // GS  # 1048576
B = 64
NCORES = 8
NGC = NG // NCORES   # 131072 groups per core
BB = 16              # width of the (o % 128) block per core
KT = IN // 128       # 64 in-tiles of 128
CK = 4               # k-tiles per chunk
NCH = KT // CK       # 16 chunks
PK0 = 2 * CK         # k-tiles per pair-group (DMA/TT grain)

_CACHE = {}


def _build_nc():
    import concourse.bacc as bacc
    import concourse.mybir as mybir
    import concourse.tile as tile
    from concourse.alu_op_type import AluOpType

    f16 = mybir.dt.bfloat16
    f32 = mybir.dt.float32
    u8 = mybir.dt.uint8

    nc = bacc.Bacc(None, target_bir_lowering=False, debug=False)

    NPG = NCH // 2  # pair-groups
    xt_d = nc.dram_tensor("xt", [128, KT * B], f16, kind="ExternalInput")
    # nibble streams laid out as contiguous per-pair-group blocks so every
    # chunk DMA reads DRAM fully sequentially (strided 4KB segments measurably
    # throttle HBM)
    hi_d = nc.dram_tensor("hi", [NPG * 128, PK0 * 512], u8, kind="ExternalInput")
    lo_d = nc.dram_tensor("lo", [NPG * 128, PK0 * 512], u8, kind="ExternalInput")
    sc_d = nc.dram_tensor("sc", [128, KT * BB], f16, kind="ExternalInput")
    sz_d = nc.dram_tensor("sz", [128, KT * BB], f16, kind="ExternalInput")
    bs_d = nc.dram_tensor("bs", [2, 512], f32, kind="ExternalInput")
    # output as [ (h t), 512 ]: rows 0:64 hi-half tokens, rows 64:128 lo-half
    y_d = nc.dram_tensor("y", [2 * B, 512], f32, kind="ExternalOutput")

    with tile.TileContext(nc) as tc:
        with (
            tc.tile_pool(name="const", bufs=1) as cpool,
            tc.tile_pool(name="wq", bufs=6) as wqpool,
            tc.tile_pool(name="nibhi", bufs=3) as hipool,
            tc.tile_pool(name="niblo", bufs=3) as lopool,
            tc.tile_pool(name="ws", bufs=3) as wspool,
            tc.tile_pool(name="psum", bufs=1, space="PSUM") as pspool,
            tc.tile_pool(name="outp", bufs=1) as opool,
        ):
            # sc leads the sync ring (ahead of the first hi chunk): the first
            # TT is co-gated by it, and on the scalar ring it crawled to
            # ~15us through the early DMA crunch
            sc_sb = cpool.tile([128, KT * BB], f16)
            nc.sync.dma_start(out=sc_sb[:], in_=sc_d[:])
            sz_sb = cpool.tile([128, KT * BB], f16)
            nc.scalar.dma_start(out=sz_sb[:], in_=sz_d[:])
            xt_sb = cpool.tile([128, KT * B], f16)
            nc.scalar.dma_start(out=xt_sb[:], in_=xt_d[:])
            bias_sb = cpool.tile([2 * B, 512], f32)

            # PE column tiling: the hi stream computes on array columns 0:63
            # (PSUM partitions 0:63), the lo stream on columns 64:127 — the
            # two N=512 matmuls per k-tile run CONCURRENTLY (tile_position is
            # auto-derived from the PSUM slice's base partition), halving the
            # tensor-engine streaming time.
            psW = pspool.tile([2 * B, 512], f32)  # rows 0:64 hi, 64:128 lo
            psC = pspool.tile([B, BB], f32)       # zero-term

            PK = PK0             # k-tiles per TT/matmul pair-group
            cw = CK * 512
            tiles = {}

            def sc_view(ka, kb):
                return (
                    sc_sb[:, ka * BB : kb * BB]
                    .rearrange("p (k b) -> p k b", b=BB)
                    .unsqueeze(2)
                    .broadcast_to((128, kb - ka, 32, BB))
                )

            def emit_tt(p, stream, spans):
                hi_f, lo_t, ws = tiles[p]
                src = hi_f if stream == "hi" else lo_t
                col0 = 0 if stream == "hi" else 512
                ws4 = ws[:].rearrange("p (k n) -> p k n", n=1024)
                for (ka, kb) in spans:
                    nc.vector.tensor_tensor(
                        out=ws4[:, ka:kb, col0 : col0 + 512].rearrange(
                            "p k (r b) -> p k r b", b=BB
                        ),
                        in0=src[:, ka * 512 : kb * 512].rearrange(
                            "p (k r b) -> p k r b", k=kb - ka, b=BB
                        ),
                        in1=sc_view(p * PK + ka, p * PK + kb),
                        op=AluOpType.mult,
                    )

            def emit_mms(p):
                ws4 = tiles[p][2][:].rearrange("p (k n) -> p k n", n=1024)
                for kl in range(PK):
                    k = p * PK + kl
                    lhsT = xt_sb[:, k * B : (k + 1) * B]
                    first = k == 0
                    last_k = k == KT - 1
                    nc.tensor.matmul(
                        psW[0:B, :], lhsT, ws4[:, kl, 0:512],
                        start=first, stop=last_k,
                    )
                    nc.tensor.matmul(
                        psW[B : 2 * B, :], lhsT, ws4[:, kl, 512:1024],
                        start=first, stop=last_k, tile_position=(0, 64),
                    )
                    nc.tensor.matmul(
                        psC[:], lhsT, sz_sb[:, k * BB : (k + 1) * BB],
                        start=first, stop=last_k,
                    )

            for pg in range(NCH // 2):
                k0 = pg * PK
                hi_f = hipool.tile([128, PK * 512], f16, tag="hi_f")
                # hi: one contiguous-block DMA per pair group on the sync
                # HWDGE ring, ScalarE casts at CK grain
                hi_u8 = wqpool.tile([128, PK * 512], u8, tag="hi_u8")
                nc.sync.dma_start(
                    out=hi_u8[:], in_=hi_d[pg * 128 : (pg + 1) * 128, :]
                )
                # first/last pair group cast at CK grain (pipeline fill /
                # short tail quantum); the steady-state middle uses one big
                # ACTIVATE per pair group — the 224-cycle per-instruction
                # overhead is what paces the hi-side mid-run
                if pg == 0 or pg == NCH // 2 - 1:
                    for half in range(2):
                        nc.scalar.activation(
                            out=hi_f[:, half * cw : (half + 1) * cw],
                            in_=hi_u8[:, half * cw : (half + 1) * cw],
                            func=mybir.ActivationFunctionType.Copy, scale=1.0,
                        )
                else:
                    nc.scalar.activation(
                        out=hi_f[:], in_=hi_u8[:],
                        func=mybir.ActivationFunctionType.Copy, scale=1.0,
                    )
                # lo: one contiguous-block SWDGE cast-DMA per pair group; the
                # LAST pair group splits at CK grain so the final lo quantum
                # (which gates the tail TT chain) lands earlier
                lo_t = lopool.tile([128, PK * 512], f16, tag="lo_f")
                lo_off = 0
                if pg == NCH // 2 - 1:
                    for half in range(2):
                        nc.gpsimd.dma_start(
                            out=lo_t[:, half * cw : (half + 1) * cw],
                            in_=lo_d[pg * 128 : (pg + 1) * 128,
                                     half * cw : (half + 1) * cw],
                        )
                else:
                    nc.gpsimd.dma_start(
                        out=lo_t[:], in_=lo_d[pg * 128 : (pg + 1) * 128, :]
                    )

                ws = wspool.tile([128, PK * 1024], f16, tag="ws")
                tiles[pg] = (hi_f, lo_t, ws)

                # pg0: TTs at CK grain so the first matmuls start sooner;
                # last pg too, so the tail quantum after the final lo chunk
                # is half as long.  (Do NOT reorder lo-TTs behind the next
                # pg's hi-TT: the DVE queue is strict FIFO, and a hi-TT
                # waiting on its ScalarE cast head-of-line-blocks the ready
                # lo-TT — measured +10us.)
                fine = pg == 0 or pg == NCH // 2 - 1
                tt_spans = [(0, CK), (CK, PK)] if fine else [(0, PK)]
                for sp in tt_spans:
                    emit_tt(pg, "hi", [sp])
                    emit_tt(pg, "lo", [sp])
                emit_mms(pg)

            # bias arrives on the gpsimd ring after the lo stream (it is only
            # needed here, ~35us in): hi-half rows 0:64, lo-half rows 64:128
            nc.gpsimd.dma_start(
                out=bias_sb[0:B, :], in_=bs_d[0:1, :].broadcast_to((B, 512))
            )
            nc.gpsimd.dma_start(
                out=bias_sb[B : 2 * B, :],
                in_=bs_d[1:2, :].broadcast_to((B, 512)),
            )

            out_sb = opool.tile([2 * B, 512], f32)
            tmp_sb = opool.tile([2 * B, 512], f32)
            psC_sb = opool.tile([2 * B, BB], f32)
            nc.scalar.copy(out=psC_sb[0:B, :], in_=psC[:])
            # the zero-term is shared by both halves (it does not depend on
            # gs); engines are partition-lockstep, so duplicate it to the
            # lo-half partitions with a tiny SBUF->SBUF DMA
            nc.sync.dma_start(out=psC_sb[B : 2 * B, :], in_=psC_sb[0:B, :])
            # tmp = psC (broadcast over g) - bias;  y = psW - tmp
            # (both on DVE: GpSimd compute steals the shared SBUF port and
            # halves the throughput of concurrent DVE tensor_tensor ops)
            nc.vector.tensor_tensor(
                out=tmp_sb[:].rearrange("p (g b) -> p g b", b=BB),
                in0=psC_sb[:].unsqueeze(1).broadcast_to((2 * B, GS // 2, BB)),
                in1=bias_sb[:].rearrange("p (g b) -> p g b", b=BB),
                op=AluOpType.subtract,
            )
            nc.vector.tensor_tensor(
                out=out_sb[:], in0=psW[:], in1=tmp_sb[:], op=AluOpType.subtract
            )
            nc.sync.dma_start(out=y_d[:], in_=out_sb[:])

    nc.compile()
    return nc


def _get_nc():
    if "nc" not in _CACHE:
        _CACHE["nc"] = _build_nc()
    return _CACHE["nc"]


def _prep_inputs(x, W_q, scale, zero, bias):
    """Host-side shard + layout prep (dtype narrowing / bit repack / transposes)."""
    xt = (
        x.T.reshape(KT, 128, B).transpose(1, 0, 2).reshape(128, KT * B)
    ).astype(ml_dtypes.bfloat16)  # (p, (k t))
    wq_u8 = W_q.astype(np.uint8)
    hi_u8 = (wq_u8 >> 4).astype(np.uint8)
    lo_u8 = (wq_u8 & 0xF).astype(np.uint8)
    sz_full = (scale.astype(np.float64) * zero.astype(np.float64)).astype(np.float32)

    def wlayout(arr_m):
        # arr_m: (32, NGC) one core's nibble slice -> contiguous per-pair-
        # group blocks [(pg p), (kl, r, b)] so each chunk DMA reads DRAM
        # sequentially
        a = arr_m.reshape(32, BB, IN)          # (r, b, in)
        a = a.transpose(2, 0, 1)               # (in, r, b): col = r*16+b
        a = a.reshape(KT, 128, 512)            # (k, p, rb)
        a = a.transpose(1, 0, 2)               # (p, k, rb)
        a = a.reshape(128, KT // PK0, PK0 * 512)  # (p, pg, cols)
        a = a.transpose(1, 0, 2)               # (pg, p, cols)
        return np.ascontiguousarray(a.reshape((KT // PK0) * 128, PK0 * 512))

    in_maps = []
    for m in range(NCORES):
        sl = slice(m * NGC, (m + 1) * NGC)
        sc_m = (
            scale[0, sl]
            .reshape(BB, IN)
            .T.reshape(KT, 128, BB)
            .transpose(1, 0, 2)
            .reshape(128, KT * BB)
        ).astype(ml_dtypes.bfloat16)
        sz_m = (
            sz_full[0, sl]
            .reshape(BB, IN)
            .T.reshape(KT, 128, BB)
            .transpose(1, 0, 2)
            .reshape(128, KT * BB)
        ).astype(ml_dtypes.bfloat16)
        # out (row h*64+t, col r*16+b)  <->  global out o = (h*32+r)*128 + m*16 + b
        bs_m = (
            bias.reshape(GS, 128)[:, m * BB : (m + 1) * BB]  # (gs, b)
            .reshape(2, 512)
            .astype(np.float32)
        )
        in_maps.append(
            {
                "xt": xt,
                "hi": wlayout(hi_u8[:, sl]),
                "lo": wlayout(lo_u8[:, sl]),
                "sc": np.ascontiguousarray(sc_m),
                "sz": np.ascontiguousarray(sz_m),
                "bs": bs_m,
            }
        )
    return in_maps


def _gather(results):
    ybig = np.stack([results[m]["y"] for m in range(NCORES)], axis=0)  # (m, 2B, 512)
    ybig = ybig.reshape(NCORES, 2, B, 32, BB)  # (m, h, t, r, b)
    return np.ascontiguousarray(
        ybig.transpose(2, 1, 3, 0, 4).reshape(B, OUT)
    )  # o = (h*32+r)*128 + m*16 + b


def run_on_hw(x, W_q, scale, zero, bias, trace=False, **trace_kw):
    """Returns (y_full, BassKernelResults)."""
    from concourse.bass_utils import run_bass_kernel_spmd

    nc = _get_nc()
    in_maps = _prep_inputs(x, W_q, scale, zero, bias)
    res = run_bass_kernel_spmd(
        nc, in_maps, list(range(NCORES)), trace=trace, **trace_kw
    )
    return _gather(res.results), res


def kernel(x, W_q, scale, zero, bias):
    y, _ = run_on_hw(x, W_q, scale, zero, bias, trace=False)
    return y


# revision 40
# speedup vs baseline: 1.1683x; 1.0132x over previous
"""HQQ-compatible 4-bit quantized linear layer on 8 Trainium2 NeuronCores.

Problem: y = x @ W.T + bias where W = ((unpack4(W_q) - zero) * scale).reshape(8192, 8192)
  x: (64, 8192) f32; W_q: (32, 1048576) int32 (bytes, two nibbles packed);
  scale/zero: (1, 1048576) f32; bias: (8192,) f32.

Math per output element (OUT=IN=8192, GS=64, NG=2**20):
  W[o, i] = (Wu[gs, ng] - zero[ng]) * scale[ng],  gs = o // 128, ng = (o % 128)*8192 + i
  Wu[r, ng] = W_q[r, ng] >> 4 (r < 32) | W_q[r-32, ng] & 0xF (r >= 32).

Sharding (tensor-parallel over output features, by ng blocks):
  core m owns ng in [m*131072, (m+1)*131072)  <=>  (o % 128) in [m*16, m*16+16).
  core m computes the 1024 outputs o = gs*128 + m*16 + b (gs in [0,64), b in [0,16)).

Per-core device pipeline (linearity: y = sum x*sc*Wu - sum x*(sc*zero) + bias):
  - host splits W_q bytes into hi/lo nibble u8 arrays (bit repacking only),
    laid out as contiguous per-pair-group DRAM blocks [(pg p), cols] so each
    chunk DMA reads DRAM sequentially
  - hi: HWDGE on the sync ring (dedicated to the 4.2MB nibble stream),
    ScalarE activation-copy casts u8 -> bf16 at half-group grain
  - lo: SWDGE (gpsimd ring) casts u8 -> bf16 in-flight, one 1MB-write
    dispatch per pair group; bias rides this ring at the tail (only needed
    at the epilogue)
  - consts (scale, scale*zero, x) ride the scalar-engine HWDGE ring; their
    dispatches overlap the wait for the first hi chunk
  - VectorE: one tensor_tensor mult per (nibble, 8-k pair group): bf16
    nibbles times scale broadcast over r (2x DVE mode; b-minor unit stride);
    first and last pair groups run at 4-k grain for faster pipeline fill and
    a shorter tail quantum
  - TensorE: per k two N=512 matmuls (hi|lo) + one N=16 matmul (sc*zero term),
    all accumulating over the 64 k-tiles in PSUM
  - epilogue: tmp = psC_bc - bias (one TT), y = psW - tmp (one TT), DMA out
    on the sync ring

Measured engine loads per core (NTFF): DVE ~39.5us (the 8.4M-element scale
multiply is its 34.2us floor at TT-bf16 2x mode), ScalarE ~33.5us, TensorE
~33us, DMA ~25MB combined at an effective 340-520 GB/s under 3-queue
contention, plus ~14us fixed framework pre/postamble inside the measured
window.  These are mutually balanced; the kernel sits at the practical
plateau of this decomposition (~70-72us).
"""

import ml_dtypes
import numpy as np

OUT = 8192
IN = 8192
GS = 64
NG = OUT * IN // GS  # 1048576
B = 64
NCORES = 8
NGC = NG // NCORES   # 131072 groups per core
BB = 16              # width of the (o % 128) block per core
KT = IN // 128       # 64 in-tiles of 128
CK = 4               # k-tiles per chunk
NCH = KT // CK       # 16 chunks
PK0 = 2 * CK         # k-tiles per pair-group (DMA/TT grain)

_CACHE = {}


def _build_nc():
    import concourse.bacc as bacc
    import concourse.mybir as mybir
    import concourse.tile as tile
    from concourse.alu_op_type import AluOpType

    f16 = mybir.dt.bfloat16
    f32 = mybir.dt.float32
    u8 = mybir.dt.uint8

    nc = bacc.Bacc(None, target_bir_lowering=False, debug=False)

    NPG = NCH // 2  # pair-groups
    xt_d = nc.dram_tensor("xt", [128, KT * B], f16, kind="ExternalInput")
    # nibble streams laid out as contiguous per-pair-group blocks so every
    # chunk DMA reads DRAM fully sequentially (strided 4KB segments measurably
    # throttle HBM)
    hi_d = nc.dram_tensor("hi", [NPG * 128, PK0 * 512], u8, kind="ExternalInput")
    lo_d = nc.dram_tensor("lo", [NPG * 128, PK0 * 512], u8, kind="ExternalInput")
    sc_d = nc.dram_tensor("sc", [128, KT * BB], f16, kind="ExternalInput")
    sz_d = nc.dram_tensor("sz", [128, KT * BB], f16, kind="ExternalInput")
    bs_d = nc.dram_tensor("bs", [2, 512], f32, kind="ExternalInput")
    # output as [ (h t), 512 ]: rows 0:64 hi-half tokens, rows 64:128 lo-half
    y_d = nc.dram_tensor("y", [2 * B, 512], f32, kind="ExternalOutput")

    with tile.TileContext(nc) as tc:
        with (
            tc.tile_pool(name="const", bufs=1) as cpool,
            tc.tile_pool(name="wq", bufs=6) as wqpool,
            tc.tile_pool(name="nibhi", bufs=3) as hipool,
            tc.tile_pool(name="niblo", bufs=3) as lopool,
            tc.tile_pool(name="ws", bufs=3) as wspool,
            tc.tile_pool(name="psum", bufs=1, space="PSUM") as pspool,
            tc.tile_pool(name="outp", bufs=1) as opool,
        ):
            # consts on the scalar-engine HWDGE ring (parallel to sync ring);
            # small ones first so they clear the ring before the 1MB xt
            sc_sb = cpool.tile([128, KT * BB], f16)
            nc.scalar.dma_start(out=sc_sb[:], in_=sc_d[:])
            sz_sb = cpool.tile([128, KT * BB], f16)
            nc.scalar.dma_start(out=sz_sb[:], in_=sz_d[:])
            xt_sb = cpool.tile([128, KT * B], f16)
            nc.scalar.dma_start(out=xt_sb[:], in_=xt_d[:])
            bias_sb = cpool.tile([2 * B, 512], f32)

            # PE column tiling: the hi stream computes on array columns 0:63
            # (PSUM partitions 0:63), the lo stream on columns 64:127 — the
            # two N=512 matmuls per k-tile run CONCURRENTLY (tile_position is
            # auto-derived from the PSUM slice's base partition), halving the
            # tensor-engine streaming time.
            psW = pspool.tile([2 * B, 512], f32)  # rows 0:64 hi, 64:128 lo
            psC = pspool.tile([B, BB], f32)       # zero-term

            PK = PK0             # k-tiles per TT/matmul pair-group
            cw = CK * 512
            tiles = {}

            def sc_view(ka, kb):
                return (
                    sc_sb[:, ka * BB : kb * BB]
                    .rearrange("p (k b) -> p k b", b=BB)
                    .unsqueeze(2)
                    .broadcast_to((128, kb - ka, 32, BB))
                )

            def emit_tt(p, stream, spans):
                hi_f, lo_t, ws = tiles[p]
                src = hi_f if stream == "hi" else lo_t
                col0 = 0 if stream == "hi" else 512
                ws4 = ws[:].rearrange("p (k n) -> p k n", n=1024)
                for (ka, kb) in spans:
                    nc.vector.tensor_tensor(
                        out=ws4[:, ka:kb, col0 : col0 + 512].rearrange(
                            "p k (r b) -> p k r b", b=BB
                        ),
                        in0=src[:, ka * 512 : kb * 512].rearrange(
                            "p (k r b) -> p k r b", k=kb - ka, b=BB
                        ),
                        in1=sc_view(p * PK + ka, p * PK + kb),
                        op=AluOpType.mult,
                    )

            def emit_mms(p):
                ws4 = tiles[p][2][:].rearrange("p (k n) -> p k n", n=1024)
                for kl in range(PK):
                    k = p * PK + kl
                    lhsT = xt_sb[:, k * B : (k + 1) * B]
                    first = k == 0
                    last_k = k == KT - 1
                    nc.tensor.matmul(
                        psW[0:B, :], lhsT, ws4[:, kl, 0:512],
                        start=first, stop=last_k,
                    )
                    nc.tensor.matmul(
                        psW[B : 2 * B, :], lhsT, ws4[:, kl, 512:1024],
                        start=first, stop=last_k, tile_position=(0, 64),
                    )
                    nc.tensor.matmul(
                        psC[:], lhsT, sz_sb[:, k * BB : (k + 1) * BB],
                        start=first, stop=last_k,
                    )

            for pg in range(NCH // 2):
                k0 = pg * PK
                hi_f = hipool.tile([128, PK * 512], f16, tag="hi_f")
                # hi: one contiguous-block DMA per pair group on the sync
                # HWDGE ring, ScalarE casts at CK grain
                hi_u8 = wqpool.tile([128, PK * 512], u8, tag="hi_u8")
                nc.sync.dma_start(
                    out=hi_u8[:], in_=hi_d[pg * 128 : (pg + 1) * 128, :]
                )
                # first/last pair group cast at CK grain (pipeline fill /
                # short tail quantum); the steady-state middle uses one big
                # ACTIVATE per pair group — the 224-cycle per-instruction
                # overhead is what paces the hi-side mid-run
                if pg == 0 or pg == NCH // 2 - 1:
                    for half in range(2):
                        nc.scalar.activation(
                            out=hi_f[:, half * cw : (half + 1) * cw],
                            in_=hi_u8[:, half * cw : (half + 1) * cw],
                            func=mybir.ActivationFunctionType.Copy, scale=1.0,
                        )
                else:
                    nc.scalar.activation(
                        out=hi_f[:], in_=hi_u8[:],
                        func=mybir.ActivationFunctionType.Copy, scale=1.0,
                    )
                # lo: one contiguous-block SWDGE cast-DMA per pair group
                lo_t = lopool.tile([128, PK * 512], f16, tag="lo_f")
                lo_off = 0
                nc.gpsimd.dma_start(
                    out=lo_t[:], in_=lo_d[pg * 128 : (pg + 1) * 128, :]
                )

                ws = wspool.tile([128, PK * 1024], f16, tag="ws")
                tiles[pg] = (hi_f, lo_t, ws)

                # pg0: TTs at CK grain so the first matmuls start sooner;
                # last pg too, so the tail quantum after the final lo chunk
                # is half as long.  (Do NOT reorder lo-TTs behind the next
                # pg's hi-TT: the DVE queue is strict FIFO, and a hi-TT
                # waiting on its ScalarE cast head-of-line-blocks the ready
                # lo-TT — measured +10us.)
                fine = pg == 0 or pg == NCH // 2 - 1
                tt_spans = [(0, CK), (CK, PK)] if fine else [(0, PK)]
                for sp in tt_spans:
                    emit_tt(pg, "hi", [sp])
                    emit_tt(pg, "lo", [sp])
                emit_mms(pg)

            # bias arrives on the gpsimd ring after the lo stream (it is only
            # needed here, ~35us in): hi-half rows 0:64, lo-half rows 64:128
            nc.gpsimd.dma_start(
                out=bias_sb[0:B, :], in_=bs_d[0:1, :].broadcast_to((B, 512))
            )
            nc.gpsimd.dma_start(
                out=bias_sb[B : 2 * B, :],
                in_=bs_d[1:2, :].broadcast_to((B, 512)),
            )

            out_sb = opool.tile([2 * B, 512], f32)
            tmp_sb = opool.tile([2 * B, 512], f32)
            psC_sb = opool.tile([2 * B, BB], f32)
            nc.scalar.copy(out=psC_sb[0:B, :], in_=psC[:])
            # the zero-term is shared by both halves (it does not depend on
            # gs); engines are partition-lockstep, so duplicate it to the
            # lo-half partitions with a tiny SBUF->SBUF DMA
            nc.sync.dma_start(out=psC_sb[B : 2 * B, :], in_=psC_sb[0:B, :])
            # tmp = psC (broadcast over g) - bias;  y = psW - tmp
            # (both on DVE: GpSimd compute steals the shared SBUF port and
            # halves the throughput of concurrent DVE tensor_tensor ops)
            nc.vector.tensor_tensor(
                out=tmp_sb[:].rearrange("p (g b) -> p g b", b=BB),
                in0=psC_sb[:].unsqueeze(1).broadcast_to((2 * B, GS // 2, BB)),
                in1=bias_sb[:].rearrange("p (g b) -> p g b", b=BB),
                op=AluOpType.subtract,
            )
            nc.vector.tensor_tensor(
                out=out_sb[:], in0=psW[:], in1=tmp_sb[:], op=AluOpType.subtract
            )
            nc.sync.dma_start(out=y_d[:], in_=out_sb[:])

    nc.compile()
    return nc


def _get_nc():
    if "nc" not in _CACHE:
        _CACHE["nc"] = _build_nc()
    return _CACHE["nc"]


def _prep_inputs(x, W_q, scale, zero, bias):
    """Host-side shard + layout prep (dtype narrowing / bit repack / transposes)."""
    xt = (
        x.T.reshape(KT, 128, B).transpose(1, 0, 2).reshape(128, KT * B)
    ).astype(ml_dtypes.bfloat16)  # (p, (k t))
    wq_u8 = W_q.astype(np.uint8)
    hi_u8 = (wq_u8 >> 4).astype(np.uint8)
    lo_u8 = (wq_u8 & 0xF).astype(np.uint8)
    sz_full = (scale.astype(np.float64) * zero.astype(np.float64)).astype(np.float32)

    def wlayout(arr_m):
        # arr_m: (32, NGC) one core's nibble slice -> contiguous per-pair-
        # group blocks [(pg p), (kl, r, b)] so each chunk DMA reads DRAM
        # sequentially
        a = arr_m.reshape(32, BB, IN)          # (r, b, in)
        a = a.transpose(2, 0, 1)               # (in, r, b): col = r*16+b
        a = a.reshape(KT, 128, 512)            # (k, p, rb)
        a = a.transpose(1, 0, 2)               # (p, k, rb)
        a = a.reshape(128, KT // PK0, PK0 * 512)  # (p, pg, cols)
        a = a.transpose(1, 0, 2)               # (pg, p, cols)
        return np.ascontiguousarray(a.reshape((KT // PK0) * 128, PK0 * 512))

    in_maps = []
    for m in range(NCORES):
        sl = slice(m * NGC, (m + 1) * NGC)
        sc_m = (
            scale[0, sl]
            .reshape(BB, IN)
            .T.reshape(KT, 128, BB)
            .transpose(1, 0, 2)
            .reshape(128, KT * BB)
        ).astype(ml_dtypes.bfloat16)
        sz_m = (
            sz_full[0, sl]
            .reshape(BB, IN)
            .T.reshape(KT, 128, BB)
            .transpose(1, 0, 2)
            .reshape(128, KT * BB)
        ).astype(ml_dtypes.bfloat16)
        # out (row h*64+t, col r*16+b)  <->  global out o = (h*32+r)*128 + m*16 + b
        bs_m = (
            bias.reshape(GS, 128)[:, m * BB : (m + 1) * BB]  # (gs, b)
            .reshape(2, 512)
            .astype(np.float32)
        )
        in_maps.append(
            {
                "xt": xt,
                "hi": wlayout(hi_u8[:, sl]),
                "lo": wlayout(lo_u8[:, sl]),
                "sc": np.ascontiguousarray(sc_m),
                "sz": np.ascontiguousarray(sz_m),
                "bs": bs_m,
            }
        )
    return in_maps


def _gather(results):
    ybig = np.stack([results[m]["y"] for m in range(NCORES)], axis=0)  # (m, 2B, 512)
    ybig = ybig.reshape(NCORES, 2, B, 32, BB)  # (m, h, t, r, b)
    return np.ascontiguousarray(
        ybig.transpose(2, 1, 3, 0, 4).reshape(B, OUT)
    )  # o = (h*32+r)*128 + m*16 + b


def run_on_hw(x, W_q, scale, zero, bias, trace=False, **trace_kw):
    """Returns (y_full, BassKernelResults)."""
    from concourse.bass_utils import run_bass_kernel_spmd

    nc = _get_nc()
    in_maps = _prep_inputs(x, W_q, scale, zero, bias)
    res = run_bass_kernel_spmd(
        nc, in_maps, list(range(NCORES)), trace=trace, **trace_kw
    )
    return _gather(res.results), res


def kernel(x, W_q, scale, zero, bias):
    y, _ = run_on_hw(x, W_q, scale, zero, bias, trace=False)
    return y
